# revision 25
# baseline (speedup 1.0000x reference)
"""Causal self-attention (B=2, T=2048, C=1024, H=16) on 8 trn2 NeuronCores.

Sharding: core c -> (batch b = c // 4, head-group g = c % 4). Each core
computes 4 heads of one batch element end-to-end (qkv slice, causal
attention, its w_proj row-block partial of the output projection).
Host sums the 4 partials per batch and adds b_proj.

Per-core dataflow (all matmul inputs bf16, PSUM accumulation fp32):
  qkT  [n=512, T]  = wqk.T @ x.T         (lhsT = wqk chunks, rhs = xT chunks)
  V    [T, 256]    = x @ wv, bias added on DVE during the vaug copy
  per q-half qh, head h, k-tile kj:
    S^T [128, W] = kT_h kj-tile vs qT_h  (K=64 contraction, causal-trimmed)
    P = exp(S^T / 8) -> bf16 sbuf, diagonal block masked by tri mask
    y~aug[65, 512] += Vaug_h[kj].T @ P   (row 64 = softmax denominator)
  y^T = y~ * broadcast(1/denom)          (fp16 selector-matmul broadcast)
  out_partial [T, 1024] bf16 = y^T-chunks as lhsT vs wp  (+ host-side bias)

vs the previous revision:
  - all DMAs are issued from the gpsimd (Pool) queue: Pool DMA dispatch is
    ~25 ns of engine time vs ~600 ns on sync/scalar, so the input stream,
    the normalize chain and the output drain no longer serialize on
    dispatch cost.
  - inputs stream in priority order (consts, wqk-h01 + xT t0 chunks, xT t1,
    wv, wqk-h23, xT t2/t3, wp) and the qkT lead-in runs chunk-major across
    four PSUM tiles, so the PE starts ~2 us in and overlaps the DMA tail.
  - wqk columns are reordered to [q01|k01|q23|k23] so the first-needed
    half streams first.
  - V bias is folded into the vaug copy (DVE add) instead of a K=1 matmul.
  - softmax reciprocals read the denominator row straight out of the y~
    SBUF tile (no SBUF->SBUF collector DMA on the critical chain) and one
    [128,512] selector matmul broadcasts a head-PAIR's reciprocals.
  - output partials are bf16, written [128,1024] per t-tile (16 output
    DMAs instead of 32 at half the bytes); host accumulates in f32.
"""

import functools
from collections import deque
from contextlib import ExitStack

import ml_dtypes
import numpy as np

import concourse.bacc as bacc
import concourse.bass as bass
import concourse.mybir as mybir
import concourse.tile as tile
from concourse import bass_utils

BF16 = mybir.dt.bfloat16
F16 = mybir.dt.float16
F32 = mybir.dt.float32
EXP = mybir.ActivationFunctionType.Exp
MULT = mybir.AluOpType.mult
ADD = mybir.AluOpType.add

T = 2048
C = 1024
HD = 64
N_CORES = 8
CCHUNK = 8    # contraction chunks of 128 over C
TT = 16       # t-tiles of 128
QC = 4        # q chunks of 512
SCALE = 1.0 / float(np.sqrt(HD))


def build_bass():
    nc = bacc.Bacc("TRN2", target_bir_lowering=False)

    xT_d = nc.dram_tensor("xT", [C, T], BF16, kind="ExternalInput").ap()
    wqk_d = nc.dram_tensor("wqk", [C, 512], BF16, kind="ExternalInput").ap()
    wv_d = nc.dram_tensor("wv", [C, 256], BF16, kind="ExternalInput").ap()
    wp_d = nc.dram_tensor("wp", [256, C], BF16, kind="ExternalInput").ap()
    bqk_d = nc.dram_tensor("bqk", [128, 4], F32, kind="ExternalInput").ap()
    bvb_d = nc.dram_tensor("bvb", [128, 256], BF16, kind="ExternalInput").ap()
    mask_d = nc.dram_tensor("mask", [128, 128], BF16, kind="ExternalInput").ap()
    sel_d = nc.dram_tensor("sel", [2, 128], F16, kind="ExternalInput").ap()
    out_d = nc.dram_tensor("out", [T, C], BF16, kind="ExternalOutput").ap()

    with tile.TileContext(nc) as tc, ExitStack() as ctx:
        const = ctx.enter_context(tc.tile_pool(name="const", bufs=1))
        xT_sb = const.tile([128, CCHUNK, T], BF16)
        wqk_sb = const.tile([128, CCHUNK, 512], BF16)
        wv_sb = const.tile([128, CCHUNK, 256], BF16)
        wp_sb = const.tile([128, 2, C], BF16)
        bqk_sb = const.tile([128, 4], F32)
        bvb_sb = const.tile([128, 256], BF16)
        mask_sb = const.tile([128, 128], BF16)
        sel_sb = const.tile([2, 128], F16)
        # per-(head-pair, q-chunk) denominator collectors and reciprocals,
        # single-partition rows: cols 512*(h%2) hold head h's 512 values
        # (partition offsets must be 32-aligned, so a [2,512] gather would
        # need a DMA; a [1,1024] row keeps everything on the DVE)
        colls = {}
        rc32s = {}
        rc16s = {}
        for hp in (0, 1):
            for qc in range(QC):
                colls[(hp, qc)] = const.tile([1, 1024], F32, name=f"coll_{hp}_{qc}")
                rc32s[(hp, qc)] = const.tile([1, 1024], F32, name=f"rc32_{hp}_{qc}")
                rc16s[(hp, qc)] = const.tile([1, 1024], F16, name=f"rc16_{hp}_{qc}")
        qkT_sb = const.tile([128, 4, T], BF16)      # q h01 | k h01 | q h23 | k h23
        vaug_sb = const.tile([128, TT, 4, 65], BF16)
        yT_sb = const.tile([128, 2, T], BF16)

        # ---- input DMA stream, priority-ordered. DMA dispatch costs
        # ~600-750 ns of engine time per transfer on EVERY queue, so spread
        # the input stream round-robin over the two queues that are idle at
        # startup (sync + gpsimd) and keep the transfer count moderate. ----
        # consts + half of wv dispatch from the scalar queue (idle until the
        # first exp ~7us in); the wqk/xT stream splits over sync + gpsimd
        nc.scalar.dma_start(bqk_sb[:, :], bqk_d[:, :])
        nc.scalar.dma_start(mask_sb[:, :], mask_d[:, :])
        nc.scalar.dma_start(sel_sb[:, :], sel_d[:, :])
        nc.scalar.dma_start(bvb_sb[:, :], bvb_d[:, :])
        qs = [nc.sync.dma_start, nc.gpsimd.dma_start]
        qi = 0

        def idma(dst, src):
            nonlocal qi
            qs[qi % 2](dst, src)
            qi += 1

        for cc in range(CCHUNK):
            c0 = 128 * cc
            idma(wqk_sb[:, cc, :], wqk_d[c0 : c0 + 128, :])
            idma(xT_sb[:, cc, 0:1024], xT_d[c0 : c0 + 128, 0:1024])
        for cc in range(4):
            c0 = 128 * cc
            nc.scalar.dma_start(wv_sb[:, cc, :], wv_d[c0 : c0 + 128, :])
        for cc in range(4, CCHUNK):
            c0 = 128 * cc
            idma(wv_sb[:, cc, :], wv_d[c0 : c0 + 128, :])
        for cc in range(CCHUNK):
            c0 = 128 * cc
            idma(xT_sb[:, cc, 1024:2048], xT_d[c0 : c0 + 128, 1024:2048])
        for dc in range(2):
            idma(wp_sb[:, dc, :], wp_d[128 * dc : 128 * dc + 128, :])
        # ones column per (t-tile, head) in the augmented-V layout
        nc.vector.memset(vaug_sb[:, :, :, 64:65], 1.0)

        def qk_ntile(pool, ni, tch):
            """qkT n-tile ni, t-chunk tch: [128, 512] of qkT + bias add.
            Pumped tiles do the bias add on the (idle) Pool engine to keep
            the DVE clear for the attention-critical ops."""
            t0 = 512 * tch
            n0 = 128 * ni
            ps = pool.tile([128, 512], F32, tag=pool.name, name=f"psqk_{ni}_{tch}")
            for cc in range(CCHUNK):
                nc.tensor.matmul(
                    ps,
                    lhsT=wqk_sb[:, cc, n0 : n0 + 128],
                    rhs=xT_sb[:, cc, t0 : t0 + 512],
                    start=(cc == 0),
                    stop=(cc == CCHUNK - 1),
                )
            nc.vector.tensor_scalar_add(
                qkT_sb[:, ni, t0 : t0 + 512], ps, bqk_sb[:, ni : ni + 1]
            )

        def v_ttile(pool, tt):
            """V t-tile tt -> vaug columns, bias added during the copy (Pool)."""
            ps = pool.tile([128, 256], F32, tag=pool.name, name=f"psv_{tt}")
            for cc in range(CCHUNK):
                nc.tensor.matmul(
                    ps,
                    lhsT=xT_sb[:, cc, 128 * tt : 128 * tt + 128],
                    rhs=wv_sb[:, cc, :],
                    start=(cc == 0),
                    stop=(cc == CCHUNK - 1),
                )
            nc.vector.tensor_tensor(
                vaug_sb[:, tt, :, 0:64],
                ps.rearrange("p (h e) -> p h e", h=4),
                bvb_sb.rearrange("p (h e) -> p h e", h=4),
                op=ADD,
            )

        # ---- phase 1 lead-in: chunk-major across 4 PSUM tiles so matmuls
        # start as soon as the first (wqk, xT) chunks land ----
        with tc.tile_pool(name="pqk", bufs=4, space="PSUM") as pqk:
            lead = [(1, 0), (1, 1), (0, 0), (0, 1)]
            pstiles = {
                nt: pqk.tile([128, 512], F32, tag="pqk", name=f"psqk_{nt[0]}_{nt[1]}")
                for nt in lead
            }
            for cc in range(CCHUNK):
                for ni, tch in lead:
                    nc.tensor.matmul(
                        pstiles[(ni, tch)],
                        lhsT=wqk_sb[:, cc, 128 * ni : 128 * ni + 128],
                        rhs=xT_sb[:, cc, 512 * tch : 512 * tch + 512],
                        start=(cc == 0),
                        stop=(cc == CCHUNK - 1),
                    )
            for ni, tch in ((1, 0), (0, 0), (0, 1), (1, 1)):
                nc.vector.tensor_scalar_add(
                    qkT_sb[:, ni, 512 * tch : 512 * tch + 512],
                    pstiles[(ni, tch)],
                    bqk_sb[:, ni : ni + 1],
                )

        # ---- attention (qh-outer), deferred-work queue pumped per k-tile ----
        with tc.tile_pool(name="expp", bufs=6) as epool, \
             tc.tile_pool(name="finp", bufs=10) as fpool, \
             tc.tile_pool(name="outp", bufs=6) as obpool, \
             ExitStack() as psum_ctx:
            spool = psum_ctx.enter_context(
                tc.tile_pool(name="ps_s", bufs=2, space="PSUM"))
            ypool = psum_ctx.enter_context(
                tc.tile_pool(name="ps_y", bufs=2, space="PSUM"))
            paux = psum_ctx.enter_context(
                tc.tile_pool(name="paux", bufs=2, space="PSUM"))

            tasks = deque()
            ysbs = {}
            bcs = {}
            obs = {}

            def pump():
                if tasks:
                    tasks.popleft()()

            def recip_half(hp, qc, r, cast_engine="vector"):
                """Reciprocal+cast for one head's [1,512] denominator half."""
                sl = slice(512 * r, 512 * r + 512)
                nc.vector.reciprocal_approx_fast(
                    rc32s[(hp, qc)][0:1, sl], colls[(hp, qc)][0:1, sl]
                )
                with nc.allow_low_precision(reason="fp16 recip for PE bcast"):
                    if cast_engine == "scalar":
                        nc.scalar.copy(
                            rc16s[(hp, qc)][0:1, sl], rc32s[(hp, qc)][0:1, sl]
                        )
                    else:
                        nc.vector.tensor_copy(
                            rc16s[(hp, qc)][0:1, sl], rc32s[(hp, qc)][0:1, sl]
                        )

            def recip_task(hp, qc):
                nc.vector.reciprocal_approx_fast(rc32s[(hp, qc)], colls[(hp, qc)])
                with nc.allow_low_precision(reason="fp16 recip for PE bcast"):
                    nc.vector.tensor_copy(rc16s[(hp, qc)], rc32s[(hp, qc)])

            def bc_task(hp, qc):
                # broadcast each head's [1,512] reciprocal row across 64
                # partitions with a K=1 ones matmul (rows 0:64 head-even,
                # 64:128 head-odd)
                bc = paux.tile([128, 512], F32, tag=paux.name, name=f"bc_{hp}_{qc}")
                for r in (0, 1):
                    nc.tensor.matmul(
                        bc[64 * r : 64 * r + 64, :],
                        lhsT=sel_sb[0:1, 0:64],
                        rhs=rc16s[(hp, qc)][0:1, 512 * r : 512 * r + 512],
                        start=True,
                        stop=True,
                    )
                bcs[(hp, qc)] = bc

            def mult_task(h, qc):
                row = h % 2
                pb = 64 * row
                bc = bcs[(h // 2, qc)]
                nc.vector.tensor_tensor(
                    yT_sb[pb : pb + 64, h // 2, 512 * qc : 512 * qc + 512],
                    ysbs[(h, qc)],
                    bc[pb : pb + 64, :],
                    op=MULT,
                )

            def proj_half(tt, nch, copy_engine="vector", pool=None, split_dma=False):
                """Half of a t-tile's projection. Merged tiles DMA once per
                t-tile; tail tiles (split_dma) DMA each half immediately so
                the final transfer finishes as soon after the last matmul as
                possible."""
                pool = pool or paux
                po = pool.tile([128, 512], F32, tag=pool.name, name=f"po_{tt}_{nch}")
                for dc in range(2):
                    nc.tensor.matmul(
                        po,
                        lhsT=yT_sb[:, dc, 128 * tt : 128 * tt + 128],
                        rhs=wp_sb[:, dc, 512 * nch : 512 * nch + 512],
                        start=(dc == 0),
                        stop=(dc == 1),
                    )
                if nch == 0:
                    obs[tt] = obpool.tile([128, C], BF16, tag="ob", name=f"ob_{tt}")
                ob = obs[tt]
                if copy_engine == "scalar":
                    nc.scalar.copy(ob[:, 512 * nch : 512 * nch + 512], po)
                else:
                    nc.vector.tensor_copy(ob[:, 512 * nch : 512 * nch + 512], po)
                if split_dma:
                    nc.sync.dma_start(
                        out_d[128 * tt : 128 * tt + 128, 512 * nch : 512 * nch + 512],
                        ob[:, 512 * nch : 512 * nch + 512],
                    )
                elif nch == 1:
                    nc.sync.dma_start(
                        out_d[128 * tt : 128 * tt + 128, :], ob
                    )

            def attn_head_half(h, qh, carry=None, on_lo_finalize=None):
                """Emit one head's attention over q-half qh. Runs `carry`
                (the previous half's trailing work) after the first k-tile's
                S^T+exp, and returns its own trailing closure."""
                pb = 64 * (h % 2)
                ni_q = 2 * (h // 2)
                ni_k = ni_q + 1
                qbase = 1024 * qh
                psy = {}
                started = {}
                remaining = {}
                for qc in (2 * qh, 2 * qh + 1):
                    psy[qc] = ypool.tile([65, 512], F32, tag="y", name=f"psy_{h}_{qc}")
                    started[qc] = False
                    remaining[qc] = 0

                def emit_y(kj, expS, qlo, qhi, off):
                    for qc in (2 * qh, 2 * qh + 1):
                        lo2 = max(qlo, 512 * qc)
                        hi2 = min(qhi, 512 * qc + 512)
                        if lo2 >= hi2:
                            continue
                        remaining[qc] -= 1
                        nc.tensor.matmul(
                            psy[qc][:, lo2 - 512 * qc : hi2 - 512 * qc],
                            lhsT=vaug_sb[:, kj, h, 0:65],
                            rhs=expS[:, off + lo2 - qlo : off + hi2 - qlo],
                            start=not started[qc],
                            stop=(remaining[qc] == 0),
                        )
                        started[qc] = True
                        if remaining[qc] == 0:
                            finalize_lite(qc)

                def finalize_lite(qc):
                    ysb = fpool.tile([64, 512], F32, tag="yf", name=f"yf_{h}_{qc}")
                    coll_slice = colls[(h // 2, qc)][
                        0:1, 512 * (h % 2) : 512 * (h % 2) + 512
                    ]
                    if h == 3 and qc == 3:
                        # the very last finalize runs after the final exp:
                        # use the now-idle ACT engine so the tail chain
                        # doesn't queue behind the DVE
                        nc.scalar.copy(ysb, psy[qc][0:64, :])
                        nc.scalar.copy(coll_slice, psy[qc][64:65, :])
                    else:
                        nc.vector.tensor_copy(ysb, psy[qc][0:64, :])
                        nc.vector.tensor_copy(coll_slice, psy[qc][64:65, :])
                    ysbs[(h, qc)] = ysb
                    if qc == 2 * qh and on_lo_finalize:
                        on_lo_finalize()

                def step(kj, expS, qlo, qhi, off):
                    emit_y(kj, expS, qlo, qhi, off)

                # bundle the causal windows into shared PSUM tiles so one
                # ACTIVATE serves several k-tiles; two-pointer packing pairs
                # big windows with small ones for near-exact 1024 fills
                # (start/stop/finalize flags derive from emission order, so
                # out-of-kj-order packing is safe)
                wins = []
                for kj in range(8 * qh + 8):
                    qlo = max(128 * kj, qbase)
                    wins.append((kj, qlo, qbase + 1024 - qlo))
                bundles = []
                if h == 0 and qh == 0:
                    # split the very first window so the first exp only
                    # needs one q-side qkT lead-in group
                    bundles = [[(0, 0, 512)], [(0, 512, 512)]]
                    wins = wins[1:]
                lo_i, hi_i = 0, len(wins) - 1
                while lo_i <= hi_i:
                    cur = [wins[lo_i]]
                    cap = wins[lo_i][2]
                    lo_i += 1
                    while lo_i <= hi_i and cap + wins[hi_i][2] <= 1024:
                        cur.append(wins[hi_i])
                        cap += wins[hi_i][2]
                        hi_i -= 1
                    bundles.append(cur)
                for bundle in bundles:
                    for kj, qlo, width in bundle:
                        for qc in (2 * qh, 2 * qh + 1):
                            if max(qlo, 512 * qc) < min(qlo + width, 512 * qc + 512):
                                remaining[qc] += 1

                pend = deque()
                first = True
                for bundle in bundles:
                    total = sum(w for _, _, w in bundle)
                    bkj = bundle[0][0]
                    ps_s = spool.tile(
                        [128, total], F32, tag="s", name=f"pss_{h}_{bkj}_{qh}"
                    )
                    off = 0
                    for kj, qlo, width in bundle:
                        qhi = qlo + width
                        a = qlo
                        while a < qhi:
                            col = off + (a - qlo)
                            stepw = min(qhi - a, 512 - (col % 512))
                            nc.tensor.matmul(
                                ps_s[:, col : col + stepw],
                                lhsT=qkT_sb[pb : pb + 64, ni_k, 128 * kj : 128 * kj + 128],
                                rhs=qkT_sb[pb : pb + 64, ni_q, a : a + stepw],
                                start=True,
                                stop=True,
                            )
                            a += stepw
                        off += width
                    expS = epool.tile(
                        [128, total], BF16, tag="es", name=f"es_{h}_{bkj}_{qh}"
                    )
                    nc.scalar.activation(expS, ps_s, EXP, scale=SCALE)
                    off = 0
                    for kj, qlo, width in bundle:
                        if qlo == 128 * kj:
                            # diagonal block: keep entries with q >= k
                            # (Pool engine: DVE is the scarcer resource)
                            nc.gpsimd.tensor_tensor(
                                expS[:, off : off + 128],
                                expS[:, off : off + 128],
                                mask_sb,
                                op=MULT,
                            )
                        off += width
                    if first and carry is not None:
                        carry()
                    else:
                        pump()
                        if qh == 0:
                            pump()
                    first = False
                    off = 0
                    for kj, qlo, width in bundle:
                        pend.append((kj, expS, qlo, qlo + width, off))
                        off += width
                        if len(pend) > 2:
                            step(*pend.popleft())

                def trailing():
                    while pend:
                        step(*pend.popleft())

                return trailing

            # qh0 deferred-work: V tiles and remaining qkT tiles, ordered so
            # each is emitted before its first consumer's head-half.
            for tt in range(0, 8):
                tasks.append(functools.partial(v_ttile, paux, tt))
            for ni, tch in ((3, 0), (3, 1), (2, 0), (2, 1)):
                tasks.append(functools.partial(qk_ntile, paux, ni, tch))
            for tt in range(8, 12):
                tasks.append(functools.partial(v_ttile, paux, tt))
            for ni, tch in ((1, 2), (1, 3), (0, 2), (0, 3)):
                tasks.append(functools.partial(qk_ntile, paux, ni, tch))
            for tt in range(12, 16):
                tasks.append(functools.partial(v_ttile, paux, tt))
            for ni, tch in ((3, 2), (3, 3), (2, 2), (2, 3)):
                tasks.append(functools.partial(qk_ntile, paux, ni, tch))

            def norm_tasks(hp, qc):
                tasks.append(functools.partial(recip_task, hp, qc))
                tasks.append(functools.partial(bc_task, hp, qc))
                tasks.append(functools.partial(mult_task, 2 * hp, qc))
                tasks.append(functools.partial(mult_task, 2 * hp + 1, qc))

            def carry_plus(prev, *fns):
                def f():
                    prev()
                    for fn in fns:
                        fn()
                return f

            carry = None
            for h in range(4):
                carry = attn_head_half(h, 0, carry)
                if h == 1:
                    norm_tasks(0, 0)   # h0/h1 qc0 done (finalized at kj4)
                elif h == 2:
                    norm_tasks(0, 1)   # h0/h1 qc1 done (h1 trailing ran in h2)
                elif h == 3:
                    norm_tasks(1, 0)

            # h3's qc1 finalize is inside its trailing; chain the recip after
            carry = carry_plus(carry, functools.partial(recip_task, 1, 1))
            tasks.append(functools.partial(bc_task, 1, 1))
            tasks.append(functools.partial(mult_task, 2, 1))
            tasks.append(functools.partial(mult_task, 3, 1))
            for tt in range(0, 8):
                for nch in range(2):
                    tasks.append(functools.partial(proj_half, tt, nch))

            def late_norm12():
                recip_task(1, 2)
                bc_task(1, 2)
                mult_task(2, 2)
                mult_task(3, 2)

            for h in range(4):
                hook = late_norm12 if h == 3 else None
                if h == 3:
                    # h2's qc3 denominator lands in h3's first bundle (the
                    # carried trailing). Appending its reciprocal half as a
                    # task FROM the carry keeps emission after the producer;
                    # only h3's half then remains on the tail chain.
                    carry = carry_plus(
                        carry,
                        lambda: tasks.append(functools.partial(recip_half, 1, 3, 0)),
                    )
                carry = attn_head_half(h, 1, carry, on_lo_finalize=hook)
                if h == 1:
                    norm_tasks(0, 2)
                elif h == 2:
                    norm_tasks(0, 3)

            # tail: finish normalize while attention psum pools still open
            carry()            # h3 qh1 trailing (y~ + finalize qc2/qc3)
            while tasks:
                tasks.popleft()()
            recip_half(1, 3, 1, cast_engine="scalar")
            bc_task(1, 3)
            mult_task(2, 3)
            mult_task(3, 3)
            psum_ctx.close()   # release s/y/aux banks for the projection

            with tc.tile_pool(name="ppo", bufs=6, space="PSUM") as popool:
                k = 0
                for tt in range(8, 16):
                    for nch in range(2):
                        proj_half(
                            tt, nch,
                            "scalar" if k % 2 == 0 else "vector",
                            pool=popool,
                            split_dma=True,
                        )
                        k += 1

    nc.compile()
    return nc


@functools.lru_cache(maxsize=1)
def _bass_cached():
    return build_bass()


def make_in_maps(x, w_attn, b_attn, w_proj):
    bf = ml_dtypes.bfloat16
    mask = np.triu(np.ones((128, 128), np.float32)).astype(bf)
    sel = np.zeros((2, 128), np.float16)
    for i in range(2):
        sel[i, 64 * i : 64 * i + 64] = 1.0
    in_maps = []
    for core in range(N_CORES):
        b, g = core // 4, core % 4
        qs = slice(256 * g, 256 * g + 256)
        ks = slice(1024 + 256 * g, 1024 + 256 * g + 256)
        vs = slice(2048 + 256 * g, 2048 + 256 * g + 256)
        wq = w_attn[:, qs]
        wk = w_attn[:, ks]
        # column order [q01 | k01 | q23 | k23]
        wqk = np.concatenate(
            [wq[:, 0:128], wk[:, 0:128], wq[:, 128:256], wk[:, 128:256]], axis=1
        ).astype(bf)
        bq = b_attn[qs]
        bk = b_attn[ks]
        bqk = np.stack(
            [bq[0:128], bk[0:128], bq[128:256], bk[128:256]], axis=0
        ).astype(np.float32)
        bvb = np.broadcast_to(
            np.asarray(b_attn[vs], np.float32)[None, :], (128, 256)
        ).astype(bf)
        in_maps.append(
            {
                "xT": np.ascontiguousarray(x[b].T).astype(bf),
                "wqk": wqk,
                "wv": np.ascontiguousarray(w_attn[:, vs]).astype(bf),
                "wp": np.ascontiguousarray(
                    w_proj[256 * g : 256 * g + 256, :]
                ).astype(bf),
                "bqk": np.ascontiguousarray(bqk.T),
                "bvb": np.ascontiguousarray(bvb),
                "mask": mask,
                "sel": sel,
            }
        )
    return in_maps


def run(x, w_attn, b_attn, w_proj, b_proj, trace=False):
    nc = _bass_cached()
    in_maps = make_in_maps(
        np.asarray(x, np.float32),
        np.asarray(w_attn, np.float32),
        np.asarray(b_attn, np.float32),
        np.asarray(w_proj, np.float32),
    )
    res = bass_utils.run_bass_kernel_spmd(
        nc, in_maps, core_ids=list(range(N_CORES)), trace=trace
    )
    out = np.zeros((2, T, C), np.float32)
    for core in range(N_CORES):
        out[core // 4] += np.asarray(res.results[core]["out"], np.float32)
    out += np.asarray(b_proj, np.float32)[None, None, :]
    return out, res


def kernel(x, w_attn, b_attn, w_proj, b_proj):
    out, _ = run(x, w_attn, b_attn, w_proj, b_proj, trace=False)
    return out


# revision 26
# speedup vs baseline: 1.2198x; 1.2198x over previous
"""Causal self-attention (B=2, T=2048, C=1024, H=16) on 8 trn2 NeuronCores.

Sharding: core c -> (batch b = c // 4, head-group g = c % 4). Each core
computes 4 heads of one batch element end-to-end (qkv slice, causal
attention, its w_proj row-block partial of the output projection).
Host sums the 4 partials per batch and adds b_proj.

Per-core dataflow (all matmul inputs bf16, PSUM accumulation fp32):
  qkT  [n=512, T]  = wqk.T @ x.T         (lhsT = wqk chunks, rhs = xT chunks)
  V    [T, 256]    = x @ wv, bias added on DVE during the vaug copy
  per q-half qh, head h, k-tile kj:
    S^T [128, W] = kT_h kj-tile vs qT_h  (K=64 contraction, causal-trimmed)
    P = exp(S^T / 8) -> bf16 sbuf, diagonal block masked by tri mask
    y~aug[65, 512] += Vaug_h[kj].T @ P   (row 64 = softmax denominator)
  y^T = y~ * broadcast(1/denom)          (fp16 selector-matmul broadcast)
  out_partial [T, 1024] bf16 = y^T-chunks as lhsT vs wp  (+ host-side bias)

vs the previous revision:
  - all DMAs are issued from the gpsimd (Pool) queue: Pool DMA dispatch is
    ~25 ns of engine time vs ~600 ns on sync/scalar, so the input stream,
    the normalize chain and the output drain no longer serialize on
    dispatch cost.
  - inputs stream in priority order (consts, wqk-h01 + xT t0 chunks, xT t1,
    wv, wqk-h23, xT t2/t3, wp) and the qkT lead-in runs chunk-major across
    four PSUM tiles, so the PE starts ~2 us in and overlaps the DMA tail.
  - wqk columns are reordered to [q01|k01|q23|k23] so the first-needed
    half streams first.
  - V bias is folded into the vaug copy (DVE add) instead of a K=1 matmul.
  - softmax reciprocals read the denominator row straight out of the y~
    SBUF tile (no SBUF->SBUF collector DMA on the critical chain) and one
    [128,512] selector matmul broadcasts a head-PAIR's reciprocals.
  - output partials are bf16, written [128,1024] per t-tile (16 output
    DMAs instead of 32 at half the bytes); host accumulates in f32.
"""

import functools
from collections import deque
from contextlib import ExitStack

import ml_dtypes
import numpy as np

import concourse.bacc as bacc
import concourse.bass as bass
import concourse.mybir as mybir
import concourse.tile as tile
from concourse import bass_utils

BF16 = mybir.dt.bfloat16
F16 = mybir.dt.float16
F32 = mybir.dt.float32
EXP = mybir.ActivationFunctionType.Exp
MULT = mybir.AluOpType.mult
ADD = mybir.AluOpType.add

T = 2048
C = 1024
HD = 64
N_CORES = 8
CCHUNK = 8    # contraction chunks of 128 over C
TT = 16       # t-tiles of 128
QC = 4        # q chunks of 512
SCALE = 1.0 / float(np.sqrt(HD))


def build_bass():
    nc = bacc.Bacc("TRN2", target_bir_lowering=False)

    xT_d = nc.dram_tensor("xT", [C, T], BF16, kind="ExternalInput").ap()
    wqk_d = nc.dram_tensor("wqk", [C, 512], BF16, kind="ExternalInput").ap()
    wv_d = nc.dram_tensor("wv", [C, 256], BF16, kind="ExternalInput").ap()
    wp_d = nc.dram_tensor("wp", [256, C], BF16, kind="ExternalInput").ap()
    bqk_d = nc.dram_tensor("bqk", [128, 4], F32, kind="ExternalInput").ap()
    bvb_d = nc.dram_tensor("bvb", [128, 256], BF16, kind="ExternalInput").ap()
    mask_d = nc.dram_tensor("mask", [128, 128], BF16, kind="ExternalInput").ap()
    sel_d = nc.dram_tensor("sel", [2, 128], F16, kind="ExternalInput").ap()
    out_d = nc.dram_tensor("out", [T, C], BF16, kind="ExternalOutput").ap()

    with tile.TileContext(nc) as tc, ExitStack() as ctx:
        const = ctx.enter_context(tc.tile_pool(name="const", bufs=1))
        xT_sb = const.tile([128, CCHUNK, T], BF16)
        wqk_sb = const.tile([128, CCHUNK, 512], BF16)
        wv_sb = const.tile([128, CCHUNK, 256], BF16)
        wp_sb = const.tile([128, 2, C], BF16)
        bqk_sb = const.tile([128, 4], F32)
        bvb_sb = const.tile([128, 256], BF16)
        mask_sb = const.tile([128, 128], BF16)
        sel_sb = const.tile([2, 128], F16)
        # per-(head-pair, q-chunk) denominator collectors and reciprocals,
        # single-partition rows: cols 512*(h%2) hold head h's 512 values
        # (partition offsets must be 32-aligned, so a [2,512] gather would
        # need a DMA; a [1,1024] row keeps everything on the DVE)
        colls = {}
        rc32s = {}
        rc16s = {}
        for hp in (0, 1):
            for qc in range(QC):
                colls[(hp, qc)] = const.tile([1, 1024], F32, name=f"coll_{hp}_{qc}")
                rc32s[(hp, qc)] = const.tile([1, 1024], F32, name=f"rc32_{hp}_{qc}")
                rc16s[(hp, qc)] = const.tile([1, 1024], F16, name=f"rc16_{hp}_{qc}")
        qkT_sb = const.tile([128, 4, T], BF16)      # q h01 | k h01 | q h23 | k h23
        vaug_sb = const.tile([128, TT, 4, 65], BF16)
        yT_sb = const.tile([128, 2, T], BF16)

        # ---- input DMA stream, priority-ordered. DMA dispatch costs
        # ~600-750 ns of engine time per transfer on EVERY queue, so spread
        # the input stream round-robin over the two queues that are idle at
        # startup (sync + gpsimd) and keep the transfer count moderate. ----
        # consts + half of wv dispatch from the scalar queue (idle until the
        # first exp ~7us in); the wqk/xT stream splits over sync + gpsimd
        nc.scalar.dma_start(bqk_sb[:, :], bqk_d[:, :])
        nc.scalar.dma_start(mask_sb[:, :], mask_d[:, :])
        nc.scalar.dma_start(sel_sb[:, :], sel_d[:, :])
        nc.scalar.dma_start(bvb_sb[:, :], bvb_d[:, :])
        qs = [nc.sync.dma_start, nc.gpsimd.dma_start]
        qi = 0

        def idma(dst, src):
            nonlocal qi
            qs[qi % 2](dst, src)
            qi += 1

        for cc in range(CCHUNK):
            c0 = 128 * cc
            idma(wqk_sb[:, cc, :], wqk_d[c0 : c0 + 128, :])
            idma(xT_sb[:, cc, 0:1024], xT_d[c0 : c0 + 128, 0:1024])
        for cc in range(4):
            c0 = 128 * cc
            nc.scalar.dma_start(wv_sb[:, cc, :], wv_d[c0 : c0 + 128, :])
        for cc in range(4, CCHUNK):
            c0 = 128 * cc
            idma(wv_sb[:, cc, :], wv_d[c0 : c0 + 128, :])
        for cc in range(CCHUNK):
            c0 = 128 * cc
            idma(xT_sb[:, cc, 1024:2048], xT_d[c0 : c0 + 128, 1024:2048])
        for dc in range(2):
            idma(wp_sb[:, dc, :], wp_d[128 * dc : 128 * dc + 128, :])
        # ones column per (t-tile, head) in the augmented-V layout
        nc.vector.memset(vaug_sb[:, :, :, 64:65], 1.0)

        def qk_ntile(pool, ni, tch):
            """qkT n-tile ni, t-chunk tch: [128, 512] of qkT + bias add.
            Pumped tiles do the bias add on the (idle) Pool engine to keep
            the DVE clear for the attention-critical ops."""
            t0 = 512 * tch
            n0 = 128 * ni
            ps = pool.tile([128, 512], F32, tag=pool.name, name=f"psqk_{ni}_{tch}")
            for cc in range(CCHUNK):
                nc.tensor.matmul(
                    ps,
                    lhsT=wqk_sb[:, cc, n0 : n0 + 128],
                    rhs=xT_sb[:, cc, t0 : t0 + 512],
                    start=(cc == 0),
                    stop=(cc == CCHUNK - 1),
                )
            nc.vector.tensor_scalar_add(
                qkT_sb[:, ni, t0 : t0 + 512], ps, bqk_sb[:, ni : ni + 1]
            )

        def v_ttile(pool, tt):
            """V t-tile tt -> vaug columns, bias added during the copy (Pool)."""
            ps = pool.tile([128, 256], F32, tag=pool.name, name=f"psv_{tt}")
            for cc in range(CCHUNK):
                nc.tensor.matmul(
                    ps,
                    lhsT=xT_sb[:, cc, 128 * tt : 128 * tt + 128],
                    rhs=wv_sb[:, cc, :],
                    start=(cc == 0),
                    stop=(cc == CCHUNK - 1),
                )
            nc.vector.tensor_tensor(
                vaug_sb[:, tt, :, 0:64],
                ps.rearrange("p (h e) -> p h e", h=4),
                bvb_sb.rearrange("p (h e) -> p h e", h=4),
                op=ADD,
            )

        # ---- phase 1 lead-in: chunk-major across 4 PSUM tiles so matmuls
        # start as soon as the first (wqk, xT) chunks land ----
        with tc.tile_pool(name="pqk", bufs=4, space="PSUM") as pqk:
            lead = [(1, 0), (1, 1), (0, 0), (0, 1)]
            pstiles = {
                nt: pqk.tile([128, 512], F32, tag="pqk", name=f"psqk_{nt[0]}_{nt[1]}")
                for nt in lead
            }
            for cc in range(CCHUNK):
                for ni, tch in lead:
                    nc.tensor.matmul(
                        pstiles[(ni, tch)],
                        lhsT=wqk_sb[:, cc, 128 * ni : 128 * ni + 128],
                        rhs=xT_sb[:, cc, 512 * tch : 512 * tch + 512],
                        start=(cc == 0),
                        stop=(cc == CCHUNK - 1),
                    )
            for ni, tch in ((1, 0), (0, 0), (0, 1), (1, 1)):
                nc.vector.tensor_scalar_add(
                    qkT_sb[:, ni, 512 * tch : 512 * tch + 512],
                    pstiles[(ni, tch)],
                    bqk_sb[:, ni : ni + 1],
                )

        # ---- attention (qh-outer), deferred-work queue pumped per k-tile ----
        with tc.tile_pool(name="expp", bufs=6) as epool, \
             tc.tile_pool(name="finp", bufs=10) as fpool, \
             tc.tile_pool(name="outp", bufs=6) as obpool, \
             ExitStack() as psum_ctx:
            spool = psum_ctx.enter_context(
                tc.tile_pool(name="ps_s", bufs=2, space="PSUM"))
            ypool = psum_ctx.enter_context(
                tc.tile_pool(name="ps_y", bufs=2, space="PSUM"))
            paux = psum_ctx.enter_context(
                tc.tile_pool(name="paux", bufs=2, space="PSUM"))

            tasks = deque()
            ysbs = {}
            bcs = {}
            obs = {}

            def pump():
                if tasks:
                    tasks.popleft()()

            def recip_half(hp, qc, r, cast_engine="vector"):
                """Reciprocal+cast for one head's [1,512] denominator half."""
                sl = slice(512 * r, 512 * r + 512)
                nc.vector.reciprocal_approx_fast(
                    rc32s[(hp, qc)][0:1, sl], colls[(hp, qc)][0:1, sl]
                )
                with nc.allow_low_precision(reason="fp16 recip for PE bcast"):
                    if cast_engine == "scalar":
                        nc.scalar.copy(
                            rc16s[(hp, qc)][0:1, sl], rc32s[(hp, qc)][0:1, sl]
                        )
                    else:
                        nc.vector.tensor_copy(
                            rc16s[(hp, qc)][0:1, sl], rc32s[(hp, qc)][0:1, sl]
                        )

            def recip_task(hp, qc):
                nc.vector.reciprocal_approx_fast(rc32s[(hp, qc)], colls[(hp, qc)])
                with nc.allow_low_precision(reason="fp16 recip for PE bcast"):
                    nc.vector.tensor_copy(rc16s[(hp, qc)], rc32s[(hp, qc)])

            def bc_task(hp, qc):
                # broadcast each head's [1,512] reciprocal row across 64
                # partitions with a K=1 ones matmul (rows 0:64 head-even,
                # 64:128 head-odd)
                bc = paux.tile([128, 512], F32, tag=paux.name, name=f"bc_{hp}_{qc}")
                for r in (0, 1):
                    nc.tensor.matmul(
                        bc[64 * r : 64 * r + 64, :],
                        lhsT=sel_sb[0:1, 0:64],
                        rhs=rc16s[(hp, qc)][0:1, 512 * r : 512 * r + 512],
                        start=True,
                        stop=True,
                    )
                bcs[(hp, qc)] = bc

            def mult_task(h, qc):
                row = h % 2
                pb = 64 * row
                bc = bcs[(h // 2, qc)]
                nc.vector.tensor_tensor(
                    yT_sb[pb : pb + 64, h // 2, 512 * qc : 512 * qc + 512],
                    ysbs[(h, qc)],
                    bc[pb : pb + 64, :],
                    op=MULT,
                )

            def proj_half(tt, nch, copy_engine="vector", pool=None, split_dma=False):
                """Half of a t-tile's projection. Merged tiles DMA once per
                t-tile; tail tiles (split_dma) DMA each half immediately so
                the final transfer finishes as soon after the last matmul as
                possible."""
                pool = pool or paux
                po = pool.tile([128, 512], F32, tag=pool.name, name=f"po_{tt}_{nch}")
                for dc in range(2):
                    nc.tensor.matmul(
                        po,
                        lhsT=yT_sb[:, dc, 128 * tt : 128 * tt + 128],
                        rhs=wp_sb[:, dc, 512 * nch : 512 * nch + 512],
                        start=(dc == 0),
                        stop=(dc == 1),
                    )
                if nch == 0:
                    obs[tt] = obpool.tile([128, C], BF16, tag="ob", name=f"ob_{tt}")
                ob = obs[tt]
                if copy_engine == "scalar":
                    nc.scalar.copy(ob[:, 512 * nch : 512 * nch + 512], po)
                else:
                    nc.vector.tensor_copy(ob[:, 512 * nch : 512 * nch + 512], po)
                if split_dma:
                    nc.sync.dma_start(
                        out_d[128 * tt : 128 * tt + 128, 512 * nch : 512 * nch + 512],
                        ob[:, 512 * nch : 512 * nch + 512],
                    )
                elif nch == 1:
                    nc.sync.dma_start(
                        out_d[128 * tt : 128 * tt + 128, :], ob
                    )

            def attn_head_half(h, qh, carry=None, on_lo_finalize=None):
                """Emit one head's attention over q-half qh. Runs `carry`
                (the previous half's trailing work) after the first k-tile's
                S^T+exp, and returns its own trailing closure."""
                pb = 64 * (h % 2)
                ni_q = 2 * (h // 2)
                ni_k = ni_q + 1
                qbase = 1024 * qh
                psy = {}
                started = {}
                remaining = {}
                for qc in (2 * qh, 2 * qh + 1):
                    psy[qc] = ypool.tile([65, 512], F32, tag="y", name=f"psy_{h}_{qc}")
                    started[qc] = False
                    remaining[qc] = 0

                def emit_y(kj, expS, qlo, qhi, off):
                    for qc in (2 * qh, 2 * qh + 1):
                        lo2 = max(qlo, 512 * qc)
                        hi2 = min(qhi, 512 * qc + 512)
                        if lo2 >= hi2:
                            continue
                        remaining[qc] -= 1
                        nc.tensor.matmul(
                            psy[qc][:, lo2 - 512 * qc : hi2 - 512 * qc],
                            lhsT=vaug_sb[:, kj, h, 0:65],
                            rhs=expS[:, off + lo2 - qlo : off + hi2 - qlo],
                            start=not started[qc],
                            stop=(remaining[qc] == 0),
                        )
                        started[qc] = True
                        if remaining[qc] == 0:
                            finalize_lite(qc)

                def finalize_lite(qc):
                    ysb = fpool.tile([64, 512], F32, tag="yf", name=f"yf_{h}_{qc}")
                    coll_slice = colls[(h // 2, qc)][
                        0:1, 512 * (h % 2) : 512 * (h % 2) + 512
                    ]
                    if h == 3 and qc == 3:
                        # the very last finalize runs after the final exp:
                        # use the now-idle ACT engine so the tail chain
                        # doesn't queue behind the DVE
                        nc.scalar.copy(ysb, psy[qc][0:64, :])
                        nc.scalar.copy(coll_slice, psy[qc][64:65, :])
                    else:
                        nc.vector.tensor_copy(ysb, psy[qc][0:64, :])
                        nc.vector.tensor_copy(coll_slice, psy[qc][64:65, :])
                    ysbs[(h, qc)] = ysb
                    if qc == 2 * qh and on_lo_finalize:
                        on_lo_finalize()

                def step(kj, expS, qlo, qhi, off):
                    emit_y(kj, expS, qlo, qhi, off)

                # bundle the causal windows into shared PSUM tiles so one
                # ACTIVATE serves several k-tiles; two-pointer packing pairs
                # big windows with small ones for near-exact 1024 fills
                # (start/stop/finalize flags derive from emission order, so
                # out-of-kj-order packing is safe)
                wins = []
                for kj in range(8 * qh + 8):
                    qlo = max(128 * kj, qbase)
                    wins.append((kj, qlo, qbase + 1024 - qlo))
                bundles = []
                if h == 0 and qh == 0:
                    # split the very first window so the first exp only
                    # needs one q-side qkT lead-in group
                    bundles = [[(0, 0, 512)], [(0, 512, 512)]]
                    wins = wins[1:]
                lo_i, hi_i = 0, len(wins) - 1
                while lo_i <= hi_i:
                    cur = [wins[lo_i]]
                    cap = wins[lo_i][2]
                    lo_i += 1
                    while lo_i <= hi_i and cap + wins[hi_i][2] <= 1024:
                        cur.append(wins[hi_i])
                        cap += wins[hi_i][2]
                        hi_i -= 1
                    bundles.append(cur)
                for bundle in bundles:
                    for kj, qlo, width in bundle:
                        for qc in (2 * qh, 2 * qh + 1):
                            if max(qlo, 512 * qc) < min(qlo + width, 512 * qc + 512):
                                remaining[qc] += 1

                pend = deque()
                first = True
                for bundle in bundles:
                    total = sum(w for _, _, w in bundle)
                    bkj = bundle[0][0]
                    ps_s = spool.tile(
                        [128, total], F32, tag="s", name=f"pss_{h}_{bkj}_{qh}"
                    )
                    off = 0
                    for kj, qlo, width in bundle:
                        qhi = qlo + width
                        a = qlo
                        while a < qhi:
                            col = off + (a - qlo)
                            stepw = min(qhi - a, 512 - (col % 512))
                            nc.tensor.matmul(
                                ps_s[:, col : col + stepw],
                                lhsT=qkT_sb[pb : pb + 64, ni_k, 128 * kj : 128 * kj + 128],
                                rhs=qkT_sb[pb : pb + 64, ni_q, a : a + stepw],
                                start=True,
                                stop=True,
                            )
                            a += stepw
                        off += width
                    expS = epool.tile(
                        [128, total], BF16, tag="es", name=f"es_{h}_{bkj}_{qh}"
                    )
                    nc.scalar.activation(expS, ps_s, EXP, scale=SCALE)
                    off = 0
                    for kj, qlo, width in bundle:
                        if qlo == 128 * kj:
                            # diagonal block: keep entries with q >= k.
                            # Stays on the DVE: this feeds the y~ matmul, and
                            # Pool's op+semaphore latency here stalls the PE
                            # often enough to knock it off its max p-state.
                            nc.vector.tensor_tensor(
                                expS[:, off : off + 128],
                                expS[:, off : off + 128],
                                mask_sb,
                                op=MULT,
                            )
                        off += width
                    if first and carry is not None:
                        carry()
                    else:
                        pump()
                        if qh == 0:
                            pump()
                    first = False
                    off = 0
                    for kj, qlo, width in bundle:
                        pend.append((kj, expS, qlo, qlo + width, off))
                        off += width
                        if len(pend) > 2:
                            step(*pend.popleft())

                def trailing():
                    while pend:
                        step(*pend.popleft())

                return trailing

            # qh0 deferred-work: V tiles and remaining qkT tiles, ordered so
            # each is emitted before its first consumer's head-half.
            for tt in range(0, 8):
                tasks.append(functools.partial(v_ttile, paux, tt))
            for ni, tch in ((3, 0), (3, 1), (2, 0), (2, 1)):
                tasks.append(functools.partial(qk_ntile, paux, ni, tch))
            for tt in range(8, 12):
                tasks.append(functools.partial(v_ttile, paux, tt))
            for ni, tch in ((1, 2), (1, 3), (0, 2), (0, 3)):
                tasks.append(functools.partial(qk_ntile, paux, ni, tch))
            for tt in range(12, 16):
                tasks.append(functools.partial(v_ttile, paux, tt))
            for ni, tch in ((3, 2), (3, 3), (2, 2), (2, 3)):
                tasks.append(functools.partial(qk_ntile, paux, ni, tch))

            def norm_tasks(hp, qc):
                tasks.append(functools.partial(recip_task, hp, qc))
                tasks.append(functools.partial(bc_task, hp, qc))
                tasks.append(functools.partial(mult_task, 2 * hp, qc))
                tasks.append(functools.partial(mult_task, 2 * hp + 1, qc))

            def carry_plus(prev, *fns):
                def f():
                    prev()
                    for fn in fns:
                        fn()
                return f

            carry = None
            for h in range(4):
                carry = attn_head_half(h, 0, carry)
                if h == 1:
                    norm_tasks(0, 0)   # h0/h1 qc0 done (finalized at kj4)
                elif h == 2:
                    norm_tasks(0, 1)   # h0/h1 qc1 done (h1 trailing ran in h2)
                elif h == 3:
                    norm_tasks(1, 0)

            # h3's qc1 finalize is inside its trailing; chain the recip after
            carry = carry_plus(carry, functools.partial(recip_task, 1, 1))
            tasks.append(functools.partial(bc_task, 1, 1))
            tasks.append(functools.partial(mult_task, 2, 1))
            tasks.append(functools.partial(mult_task, 3, 1))
            for tt in range(0, 8):
                for nch in range(2):
                    tasks.append(functools.partial(proj_half, tt, nch))

            def late_norm12():
                recip_task(1, 2)
                bc_task(1, 2)
                mult_task(2, 2)
                mult_task(3, 2)

            for h in range(4):
                hook = late_norm12 if h == 3 else None
                if h == 3:
                    # h2's qc3 denominator lands in h3's first bundle (the
                    # carried trailing). Appending its reciprocal half as a
                    # task FROM the carry keeps emission after the producer;
                    # only h3's half then remains on the tail chain.
                    carry = carry_plus(
                        carry,
                        lambda: tasks.append(functools.partial(recip_half, 1, 3, 0)),
                    )
                carry = attn_head_half(h, 1, carry, on_lo_finalize=hook)
                if h == 1:
                    norm_tasks(0, 2)
                elif h == 2:
                    norm_tasks(0, 3)

            # tail: finish normalize while attention psum pools still open
            carry()            # h3 qh1 trailing (y~ + finalize qc2/qc3)
            while tasks:
                tasks.popleft()()
            recip_half(1, 3, 1, cast_engine="scalar")
            bc_task(1, 3)
            mult_task(2, 3)
            mult_task(3, 3)
            psum_ctx.close()   # release s/y/aux banks for the projection

            with tc.tile_pool(name="ppo", bufs=6, space="PSUM") as popool:
                k = 0
                for tt in range(8, 16):
                    for nch in range(2):
                        proj_half(
                            tt, nch,
                            "scalar" if k % 2 == 0 else "vector",
                            pool=popool,
                            split_dma=True,
                        )
                        k += 1

    nc.compile()
    return nc


@functools.lru_cache(maxsize=1)
def _bass_cached():
    return build_bass()


def make_in_maps(x, w_attn, b_attn, w_proj):
    bf = ml_dtypes.bfloat16
    mask = np.triu(np.ones((128, 128), np.float32)).astype(bf)
    sel = np.zeros((2, 128), np.float16)
    for i in range(2):
        sel[i, 64 * i : 64 * i + 64] = 1.0
    in_maps = []
    for core in range(N_CORES):
        b, g = core // 4, core % 4
        qs = slice(256 * g, 256 * g + 256)
        ks = slice(1024 + 256 * g, 1024 + 256 * g + 256)
        vs = slice(2048 + 256 * g, 2048 + 256 * g + 256)
        wq = w_attn[:, qs]
        wk = w_attn[:, ks]
        # column order [q01 | k01 | q23 | k23]
        wqk = np.concatenate(
            [wq[:, 0:128], wk[:, 0:128], wq[:, 128:256], wk[:, 128:256]], axis=1
        ).astype(bf)
        bq = b_attn[qs]
        bk = b_attn[ks]
        bqk = np.stack(
            [bq[0:128], bk[0:128], bq[128:256], bk[128:256]], axis=0
        ).astype(np.float32)
        bvb = np.broadcast_to(
            np.asarray(b_attn[vs], np.float32)[None, :], (128, 256)
        ).astype(bf)
        in_maps.append(
            {
                "xT": np.ascontiguousarray(x[b].T).astype(bf),
                "wqk": wqk,
                "wv": np.ascontiguousarray(w_attn[:, vs]).astype(bf),
                "wp": np.ascontiguousarray(
                    w_proj[256 * g : 256 * g + 256, :]
                ).astype(bf),
                "bqk": np.ascontiguousarray(bqk.T),
                "bvb": np.ascontiguousarray(bvb),
                "mask": mask,
                "sel": sel,
            }
        )
    return in_maps


def run(x, w_attn, b_attn, w_proj, b_proj, trace=False):
    nc = _bass_cached()
    in_maps = make_in_maps(
        np.asarray(x, np.float32),
        np.asarray(w_attn, np.float32),
        np.asarray(b_attn, np.float32),
        np.asarray(w_proj, np.float32),
    )
    res = bass_utils.run_bass_kernel_spmd(
        nc, in_maps, core_ids=list(range(N_CORES)), trace=trace
    )
    out = np.zeros((2, T, C), np.float32)
    for core in range(N_CORES):
        out[core // 4] += np.asarray(res.results[core]["out"], np.float32)
    out += np.asarray(b_proj, np.float32)[None, None, :]
    return out, res


def kernel(x, w_attn, b_attn, w_proj, b_proj):
    out, _ = run(x, w_attn, b_attn, w_proj, b_proj, trace=False)
    return out
